# revision 6
# baseline (speedup 1.0000x reference)
"""Trainium2 Bass kernel for nn_CrossTalk (segment scatter-add -> tridiag mix -> gather).

Full (unsharded) inputs in, full output out. Internally shards the wavelength
axis of flux across 8 NeuronCores (512 rows each), and on each core runs a
Tile/Bass kernel that processes four 128-row chunks:

  per 128x7200 chunk (rows = wavelengths on partitions, positions on free axis):
    G1   gpsimd.ap_gather   : sort flux columns by segment id (host-computed perm)
    ACT  scaled copy        : S *= (1 - 2*eta)
    DVE  tensor_tensor_scan : segmented prefix-sum (mask kills state at run starts)
                              -> binned sums sit at run-end columns of B
    G2   gpsimd.ap_gather   : scatter run-end values to a per-tile-padded natural
                              bin layout N (empty bins / pads index B's zero col)
    DVE  tt_add + stt       : tridiagonal mix M = C + eta' * (L + R)
    G3   gpsimd.ap_gather   : gather M back to original positions
    DMA  out

All index metadata (sort permutation, run ends, segment ids) is tiny host-side
precompute from tile_idx/fib_idx; the flux data is only touched on-device.
"""

import os
import sys

import numpy as np

for _p in ("/opt/trn_rl_repo", "/root/.axon_site/_ro/trn_rl_repo"):
    if os.path.isdir(_p) and _p not in sys.path:
        sys.path.insert(0, _p)

import ml_dtypes  # noqa: E402

import concourse.bacc as bacc  # noqa: E402
import concourse.mybir as mybir  # noqa: E402
from concourse.tile import TileContext  # noqa: E402
from concourse.bass_utils import run_bass_kernel_spmd  # noqa: E402

# Problem geometry (fixed by the harness spec).
N_WAVE = 4096
N_TILES = 12
N_FIBRES = 600
N_POS = N_TILES * N_FIBRES          # 7200
N_CORES = 8
ROWS_PER_CORE = N_WAVE // N_CORES   # 512
P = 128                             # SBUF partitions
N_CHUNKS = ROWS_PER_CORE // P       # 4
PAD_W = N_FIBRES + 4                # 604: per-tile padded width (zeros at 0,601..603)
NAT_W = N_TILES * PAD_W             # 7248 (multiple of 16)

F32 = mybir.dt.float32
BF16 = mybir.dt.bfloat16
I16 = mybir.dt.int16

_PROGRAM_CACHE = {}


def _wrap_idx(flat):
    """ap_gather index layout: idxs[p, s] = flat[s*16 + p], tiled to 128 partitions."""
    flat = np.asarray(flat, np.int64)
    assert flat.size % 16 == 0
    w = flat.reshape(flat.size // 16, 16).T.astype(np.int16)   # [16, S]
    return np.tile(w, (P // 16, 1))                            # [128, S]


def _host_precompute(tile_idx, fib_idx):
    seg = (tile_idx.astype(np.int64) * N_FIBRES + fib_idx.astype(np.int64)).astype(np.int64)
    order = np.argsort(seg, kind="stable")
    sseg = seg[order]

    mask = np.zeros(N_POS, np.float32)
    mask[1:] = (sseg[1:] == sseg[:-1]).astype(np.float32)

    is_end = np.ones(N_POS, bool)
    is_end[:-1] = sseg[1:] != sseg[:-1]
    end_j = np.nonzero(is_end)[0]
    endcol = np.zeros(N_POS, np.int64)          # empty bins -> B column 0 (zero)
    endcol[sseg[end_j]] = end_j + 1             # B has a leading zero column

    nat = np.zeros(NAT_W, np.int64)
    nat.reshape(N_TILES, PAD_W)[:, 1:N_FIBRES + 1] = endcol.reshape(N_TILES, N_FIBRES)

    return {
        "sort_idx": _wrap_idx(order),
        "nat_idx": _wrap_idx(nat),
        "out_idx": _wrap_idx(seg),
        "scan_mask": np.tile(mask.astype(ml_dtypes.bfloat16)[None, :], (P, 1)),
    }


def _build_program():
    key = "v1"
    if key in _PROGRAM_CACHE:
        return _PROGRAM_CACHE[key]

    nc = bacc.Bacc("TRN2", target_bir_lowering=False, debug=False)

    flux_d = nc.dram_tensor("flux", [ROWS_PER_CORE, N_POS], F32, kind="ExternalInput").ap()
    sort_d = nc.dram_tensor("sort_idx", [P, N_POS // 16], I16, kind="ExternalInput").ap()
    nat_d = nc.dram_tensor("nat_idx", [P, NAT_W // 16], I16, kind="ExternalInput").ap()
    oidx_d = nc.dram_tensor("out_idx", [P, N_POS // 16], I16, kind="ExternalInput").ap()
    mask_d = nc.dram_tensor("scan_mask", [P, N_POS], BF16, kind="ExternalInput").ap()
    consts_d = nc.dram_tensor("consts", [P, 2], F32, kind="ExternalInput").ap()
    out_d = nc.dram_tensor("out", [ROWS_PER_CORE, N_POS], F32, kind="ExternalOutput").ap()

    mult = mybir.AluOpType.mult
    add = mybir.AluOpType.add
    Copy = mybir.ActivationFunctionType.Copy

    with TileContext(nc) as tc:
        with (
            tc.tile_pool(name="pers", bufs=1) as pers,
            tc.tile_pool(name="fo", bufs=2) as fo_pool,
            tc.tile_pool(name="sw", bufs=2) as sw_pool,
            tc.tile_pool(name="nn", bufs=1) as n_pool,
            tc.tile_pool(name="mm", bufs=1) as m_pool,
        ):
            # ---- one-time setup ----
            sort_t = pers.tile([P, N_POS // 16], I16, tag="sidx")
            nat_t = pers.tile([P, NAT_W // 16], I16, tag="nidx")
            oidx_t = pers.tile([P, N_POS // 16], I16, tag="oidx")
            mask_t = pers.tile([P, N_POS], BF16, tag="mask")
            consts_t = pers.tile([P, 2], F32, tag="consts")
            nc.sync.dma_start(sort_t[:], sort_d)
            nc.sync.dma_start(nat_t[:], nat_d)
            nc.sync.dma_start(oidx_t[:], oidx_d)
            nc.sync.dma_start(mask_t[:], mask_d)
            nc.sync.dma_start(consts_t[:], consts_d)
            c0_ap = consts_t[:, 0:1]     # 1 - 2*eta
            etap_ap = consts_t[:, 1:2]   # eta / (1 - 2*eta)

            F = {}
            S = {}
            N = {}
            M = {}
            O = {}

            def dma_in(i):
                F[i] = fo_pool.tile([P, N_POS], F32, tag="FO", name=f"F{i}")
                nc.sync.dma_start(F[i][:], flux_d[P * i:P * (i + 1), :])

            def front(i):  # G1 + scale + scan (scan runs in place in S cols 1:)
                S[i] = sw_pool.tile([P, N_POS + 1], F32, tag="SW", name=f"S{i}")
                sv = S[i][:, 1:N_POS + 1]
                nc.gpsimd.ap_gather(sv, F[i][:], sort_t[:],
                                    channels=P, num_elems=N_POS, d=1, num_idxs=N_POS)
                nc.scalar.memzero(S[i][:, 0:1])
                nc.scalar.activation(sv, sv, Copy, scale=c0_ap)
                nc.vector.tensor_tensor_scan(sv, mask_t[:], sv, 0.0, mult, add)

            def middle(i):  # G2 + tridiag mix
                N[i] = n_pool.tile([P, NAT_W], F32, tag="N", name=f"N{i}")
                nc.gpsimd.ap_gather(N[i][:], S[i][:], nat_t[:],
                                    channels=P, num_elems=N_POS + 1, d=1, num_idxs=NAT_W)
                nv = N[i][:].rearrange("p (t w) -> p t w", w=PAD_W)
                T1 = sw_pool.tile([P, N_POS], F32, tag="SW", name=f"T1_{i}")
                t1v = T1[:].rearrange("p (t w) -> p t w", w=N_FIBRES)
                nc.vector.tensor_tensor(
                    t1v, nv[:, :, 0:N_FIBRES], nv[:, :, 2:N_FIBRES + 2], add)
                M[i] = m_pool.tile([P, N_POS], F32, tag="M", name=f"M{i}")
                mv = M[i][:].rearrange("p (t w) -> p t w", w=N_FIBRES)
                nc.vector.scalar_tensor_tensor(
                    mv, t1v, etap_ap, nv[:, :, 1:N_FIBRES + 1], mult, add)

            def back(i):  # G3 + store
                O[i] = fo_pool.tile([P, N_POS], F32, tag="FO", name=f"O{i}")
                nc.gpsimd.ap_gather(O[i][:], M[i][:], oidx_t[:],
                                    channels=P, num_elems=N_POS, d=1, num_idxs=N_POS)
                nc.sync.dma_start(out_d[P * i:P * (i + 1), :], O[i][:])

            # software-pipelined emission: GPSIMD stream per step t is
            # [G1(t), G2(t-1), G3(t-2)] so the (bottleneck) gather engine
            # never sits behind a same-chunk DVE dependency.
            dma_in(0)
            if N_CHUNKS > 1:
                dma_in(1)
            for t in range(N_CHUNKS + 2):
                if t < N_CHUNKS:
                    front(t)
                    if t + 2 < N_CHUNKS:
                        dma_in(t + 2)
                if 0 <= t - 1 < N_CHUNKS:
                    middle(t - 1)
                if 0 <= t - 2 < N_CHUNKS:
                    back(t - 2)

    nc.compile()
    _PROGRAM_CACHE[key] = nc
    return nc


def _run(inputs, trace=False, trace_kwargs=None):
    flux = np.ascontiguousarray(np.asarray(inputs["flux"], dtype=np.float32))
    eta = float(np.asarray(inputs["eta"]).reshape(-1)[0])
    tile_idx = np.asarray(inputs["tile_idx"]).astype(np.int64).reshape(-1)
    fib_idx = np.asarray(inputs["fib_idx"]).astype(np.int64).reshape(-1)

    assert flux.shape == (N_WAVE, N_POS), flux.shape
    assert tile_idx.shape == (N_POS,) and fib_idx.shape == (N_POS,)

    aux = _host_precompute(tile_idx, fib_idx)
    c0 = np.float32(1.0 - 2.0 * eta)
    etap = np.float32(eta / float(c0))
    consts = np.tile(np.array([[c0, etap]], np.float32), (P, 1))

    nc = _build_program()

    shared = {
        "sort_idx": aux["sort_idx"],
        "nat_idx": aux["nat_idx"],
        "out_idx": aux["out_idx"],
        "scan_mask": aux["scan_mask"],
        "consts": consts,
    }
    in_maps = []
    for c in range(N_CORES):
        m = dict(shared)
        m["flux"] = np.ascontiguousarray(
            flux[c * ROWS_PER_CORE:(c + 1) * ROWS_PER_CORE])
        in_maps.append(m)

    kwargs = {}
    if trace:
        kwargs["trace"] = True
        if trace_kwargs:
            kwargs["trace_kwargs"] = trace_kwargs
    res = run_bass_kernel_spmd(nc, in_maps, core_ids=list(range(N_CORES)), **kwargs)
    out = np.concatenate([r["out"] for r in res.results], axis=0)
    return out, res


def kernel(**inputs) -> np.ndarray:
    out, _ = _run(inputs, trace=False)
    return out
